# revision 11
# baseline (speedup 1.0000x reference)
"""Trainium2 Bass kernel for AggregationRebuild (GNN message passing).

Reference computation (N=2048, K=8, L=64, D=64):
    sub_sim[n,k]  = sim[n, index[n,k]] / 0.5
    W             = softmax(sub_sim, axis=-1)                 # [N, K]
    out[n]        = sum_k W[n,k] * emb[index[n,k]]            # [N, L*D]

Sharding: rows split across 8 cores (256 rows each); emb replicated.

Per-core program (2 row-chunks of 128 = partition dim):
  - dma_gather 256B blocks of the core's sim slab that contain each
    sub-similarity element; extract the element with an iota==rmod one-hot
    (tensor_scalar) fused multiply+reduce (tensor_tensor_reduce, scale=2
    folds the /T).
  - softmax over K on ACT (Exp with per-partition -max bias, fused sum).
  - dma_gather each neighbor's 16KB feature row into its own partition;
    weighted sum via diagonal matmuls accumulating in PSUM
    (psum += diag(W[:,k]) @ F_k), then ACT copy PSUM->SBUF and DMA out.
"""

import sys
import types

import numpy as np

import concourse.bass as bass
import concourse.tile as tile
from concourse import bacc, library_config, mybir
from concourse.bass_utils import run_bass_kernel_spmd


def _install_axon_ntff_hook():
    """The agent image's antenv lacks axon_hooks; synthesize it so
    run_bass_kernel_spmd(trace=True) can capture NTFF profiles."""
    try:
        from antenv.axon_hooks import get_axon_ntff_profile_hook  # noqa: F401
        return
    except ImportError:
        pass
    try:
        from trn_agent_boot.trn_boot import _ntff_profile_via_ctypes
        hook = _ntff_profile_via_ctypes("/opt/axon/libaxon_pjrt.so")
    except Exception:
        hook = None
    mod = types.ModuleType("antenv.axon_hooks")
    mod.get_axon_ntff_profile_hook = lambda: hook
    mod.set_axon_ntff_profile_hook = lambda h: None
    sys.modules["antenv.axon_hooks"] = mod

F32 = mybir.dt.float32
I16 = mybir.dt.int16

N, K, L, D = 2048, 8, 64, 64
LD = L * D                      # 4096
NCORES = 8
RPD = N // NCORES               # 256 rows per device
NCH = RPD // 128                # 2 chunks of 128 rows
SBLK = 64                       # sim gather block (elements) = 256B
NBLK = N // SBLK                # 32 blocks per sim row
MM_N = 512                      # matmul free-dim (one PSUM bank)
F_BUFS = 5
O_BUFS = 2
B_BUFS = 8
USE_TTR = False  # InstTensorTensorReduce faults the exec unit on this HW

_cache: dict = {}


def build(variant="full"):
    nc = bacc.Bacc("TRN2", target_bir_lowering=False, debug=False,
                   num_devices=NCORES)

    emb = nc.dram_tensor("emb", [N, LD], F32, kind="ExternalInput")
    simb = nc.dram_tensor("simb", [RPD * NBLK, SBLK], F32, kind="ExternalInput")
    gidx = nc.dram_tensor("gidx", [128, 2 * NCH * K, 8], I16, kind="ExternalInput")
    rmod = nc.dram_tensor("rmod", [128, NCH * K], F32, kind="ExternalInput")
    iotac = nc.dram_tensor("iotac", [128, SBLK], F32, kind="ExternalInput")
    ident = nc.dram_tensor("ident", [128, 128], F32, kind="ExternalInput")
    wout = nc.dram_tensor("wout", [RPD, K], F32, kind="ExternalOutput")
    eout = nc.dram_tensor("eout", [RPD, LD], F32, kind="ExternalOutput")

    with tile.TileContext(nc) as tc:
        with (
            tc.tile_pool(name="const", bufs=1) as cpool,
            tc.tile_pool(name="bblk", bufs=B_BUFS) as bpool,
            tc.tile_pool(name="hh", bufs=2) as hpool,
            tc.tile_pool(name="sm", bufs=2) as smpool,
            tc.tile_pool(name="stat", bufs=8) as stpool,
            tc.tile_pool(name="wd", bufs=3) as dpool,
            tc.tile_pool(name="feat", bufs=F_BUFS) as fpool,
            tc.tile_pool(name="out", bufs=O_BUFS) as opool,
            tc.tile_pool(name="ps", bufs=1, space="PSUM") as pspool,
        ):
            gidx_t = cpool.tile([128, 2 * NCH * K, 8], I16)
            nc.sync.dma_start(gidx_t[:], gidx[:])
            rmod_t = cpool.tile([128, NCH * K], F32)
            nc.sync.dma_start(rmod_t[:], rmod[:])
            iota_t = cpool.tile([128, SBLK], F32)
            nc.sync.dma_start(iota_t[:], iotac[:])
            ident_t = cpool.tile([128, 128], F32)
            nc.sync.dma_start(ident_t[:], ident[:])

            nc.gpsimd.load_library(library_config.mlp)

            for c in range(NCH):
                # --- gathers (sim blocks paired with feature rows) ---
                bts, fts = [], []
                for k in range(K):
                    bt = bpool.tile([128, 1, SBLK], F32)
                    nc.gpsimd.dma_gather(
                        bt[:], simb[:], gidx_t[:, (NCH + c) * K + k, :],
                        128, 128, SBLK)
                    bts.append(bt)
                    ft = fpool.tile([128, 1, LD], F32)
                    nc.gpsimd.dma_gather(
                        ft[:], emb[:], gidx_t[:, c * K + k, :],
                        128, 128, LD)
                    fts.append(ft)

                # --- sub-similarity extraction ---
                sub = smpool.tile([128, K], F32)
                if variant in ("full", "nomm", "x1", "x2"):
                    for k in range(K):
                        j = c * K + k
                        h = hpool.tile([128, SBLK], F32, tag="h")
                        if USE_TTR:
                            nc.vector.tensor_scalar(
                                out=h[:], in0=iota_t[:],
                                scalar1=rmod_t[:, j:j + 1],
                                scalar2=None, op0=mybir.AluOpType.is_equal)
                        else:
                            # h = (iota == rmod) * 2  (folds the 1/T scale)
                            nc.vector.tensor_scalar(
                                out=h[:], in0=iota_t[:],
                                scalar1=rmod_t[:, j:j + 1],
                                scalar2=2.0, op0=mybir.AluOpType.is_equal,
                                op1=mybir.AluOpType.mult)
                        if USE_TTR:
                            p = hpool.tile([128, SBLK], F32, tag="p")
                            nc.vector.tensor_tensor_reduce(
                                out=p[:], in0=h[:], in1=bts[k][:, 0, :],
                                scale=2.0, scalar=0.0,
                                op0=mybir.AluOpType.mult,
                                op1=mybir.AluOpType.add,
                                accum_out=sub[:, k:k + 1])
                        else:
                            p = hpool.tile([128, SBLK], F32, tag="p")
                            nc.vector.tensor_tensor(
                                out=p[:], in0=h[:], in1=bts[k][:, 0, :],
                                op=mybir.AluOpType.mult)
                            nc.vector.reduce_sum(
                                out=sub[:, k:k + 1], in_=p[:],
                                axis=mybir.AxisListType.X)
                else:
                    nc.vector.memset(sub[:], 0.125)

                # --- softmax over K ---
                w = smpool.tile([128, K], F32, tag="w")
                if variant == "x1":
                    nc.vector.tensor_copy(w[:], sub[:])
                elif variant == "x2":
                    negmx = stpool.tile([128, 1], F32, tag="negmx")
                    nc.vector.reduce_max(out=negmx[:], in_=sub[:],
                                         axis=mybir.AxisListType.X, negate=True)
                    e = smpool.tile([128, K], F32, tag="e")
                    ssum = stpool.tile([128, 1], F32, tag="ssum")
                    nc.scalar.activation(
                        out=e[:], in_=sub[:],
                        func=mybir.ActivationFunctionType.Exp,
                        bias=negmx[:], scale=1.0, accum_out=ssum[:])
                    nc.vector.tensor_copy(w[:], e[:])
                elif variant in ("full", "nomm"):
                    negmx = stpool.tile([128, 1], F32, tag="negmx")
                    nc.vector.reduce_max(out=negmx[:], in_=sub[:],
                                         axis=mybir.AxisListType.X, negate=True)
                    e = smpool.tile([128, K], F32, tag="e")
                    ssum = stpool.tile([128, 1], F32, tag="ssum")
                    nc.scalar.activation(
                        out=e[:], in_=sub[:],
                        func=mybir.ActivationFunctionType.Exp,
                        bias=negmx[:], scale=1.0, accum_out=ssum[:])
                    rcp = stpool.tile([128, 1], F32, tag="rcp")
                    nc.vector.reciprocal(rcp[:], ssum[:])
                    nc.vector.tensor_scalar(
                        out=w[:], in0=e[:], scalar1=rcp[:, 0:1], scalar2=None,
                        op0=mybir.AluOpType.mult)
                else:
                    nc.vector.memset(w[:], 0.125)
                nc.sync.dma_start(wout[c * 128:(c + 1) * 128, :], w[:])

                # --- weighted sum of neighbor features in PSUM ---
                if variant in ("full", "nosm"):
                    psum = pspool.tile([128, LD], F32)
                    for k in range(K):
                        wd = dpool.tile([128, 128], F32)
                        nc.vector.tensor_scalar(
                            out=wd[:], in0=ident_t[:], scalar1=w[:, k:k + 1],
                            scalar2=None, op0=mybir.AluOpType.mult)
                        for s in range(LD // MM_N):
                            nc.tensor.matmul(
                                out=psum[:, s * MM_N:(s + 1) * MM_N],
                                lhsT=wd[:],
                                rhs=fts[k][:, 0, s * MM_N:(s + 1) * MM_N],
                                start=(k == 0), stop=(k == K - 1))

                    o = opool.tile([128, LD], F32)
                    nc.scalar.copy(o[:], psum[:])
                    nc.sync.dma_start(eout[c * 128:(c + 1) * 128, :], o[:])

    nc.compile()
    return nc


def prep_inputs(similarity_matrix, batch_emb_om, index):
    """Shard + marshal full inputs into per-core input maps."""
    sim = np.ascontiguousarray(similarity_matrix, dtype=np.float32)
    emb = np.ascontiguousarray(
        np.asarray(batch_emb_om, dtype=np.float32).reshape(N, LD))
    idx = np.asarray(index, dtype=np.int64).astype(np.int32)

    iotac = np.tile(np.arange(SBLK, dtype=np.float32), (128, 1))
    identc = np.eye(128, dtype=np.float32)

    in_maps = []
    for d in range(NCORES):
        idx_d = idx[d * RPD:(d + 1) * RPD]            # [256, K]
        gidx_d = np.zeros((128, 2 * NCH * K, 8), dtype=np.int16)
        rmod_d = np.zeros((128, NCH * K), dtype=np.float32)
        for c in range(NCH):
            rows = np.arange(c * 128, (c + 1) * 128)
            for k in range(K):
                cols = idx_d[rows, k]                  # [128]
                feat_lst = cols.astype(np.int16)
                sim_lst = (rows * NBLK + (cols >> 6)).astype(np.int16)
                # wrap [i] -> [i%16, i//16], replicate to 128 partitions
                gidx_d[:, c * K + k, :] = np.tile(
                    feat_lst.reshape(8, 16).T, (8, 1))
                gidx_d[:, (NCH + c) * K + k, :] = np.tile(
                    sim_lst.reshape(8, 16).T, (8, 1))
                rmod_d[:, c * K + k] = (cols & 63).astype(np.float32)
        in_maps.append({
            "emb": emb,
            "simb": sim[d * RPD:(d + 1) * RPD].reshape(RPD * NBLK, SBLK),
            "gidx": gidx_d,
            "rmod": rmod_d,
            "iotac": iotac,
            "ident": identc,
        })
    return in_maps


def assemble_outputs(results):
    wfull = np.concatenate([r["wout"] for r in results], axis=0)
    efull = np.concatenate([r["eout"] for r in results], axis=0)
    return wfull.reshape(N, K), efull.reshape(N, L, D)


def kernel(similarity_matrix, batch_emb_om, index, trace=False):
    if trace:
        _install_axon_ntff_hook()
    if "nc" not in _cache:
        _cache["nc"] = build()
    nc = _cache["nc"]
    in_maps = prep_inputs(similarity_matrix, batch_emb_om, index)
    res = run_bass_kernel_spmd(nc, in_maps, core_ids=list(range(NCORES)),
                               trace=trace)
    out = assemble_outputs(res.results)
    if trace:
        _cache["last_result"] = res
    return out


# revision 12
# speedup vs baseline: 1.4013x; 1.4013x over previous
"""Trainium2 Bass kernel for AggregationRebuild (GNN message passing).

Reference computation (N=2048, K=8, L=64, D=64):
    sub_sim[n,k]  = sim[n, index[n,k]] / 0.5
    W             = softmax(sub_sim, axis=-1)                 # [N, K]
    out[n]        = sum_k W[n,k] * emb[index[n,k]]            # [N, L*D]

Sharding: rows split across 8 cores (256 rows each); emb replicated.

Per-core program (2 row-chunks of 128 rows = partition dim):
  - dma_gather 256B blocks of the core's sim slab containing each
    sub-similarity element; extract with an iota==rmod one-hot
    (tensor_scalar is_equal*2 folds the /T), multiply + free-axis reduce.
  - softmax over K on ACT (Exp with per-partition -max bias, fused sum).
  - dma_gather each neighbor's 16KB feature row into its own partition;
    weighted sum: ACT multiplies by the per-partition softmax weight
    (activation Copy with scale AP), DVE accumulates. All fp32.
"""

import sys
import types

import numpy as np

import concourse.bass as bass
import concourse.tile as tile
from concourse import bacc, library_config, mybir
from concourse.bass_utils import run_bass_kernel_spmd

F32 = mybir.dt.float32
I16 = mybir.dt.int16

N, K, L, D = 2048, 8, 64, 64
LD = L * D                      # 4096
NCORES = 8
RPD = N // NCORES               # 256 rows per device
NCH = RPD // 128                # 2 chunks of 128 rows
SBLK = 64                       # sim gather block (elements) = 256B
NBLK = N // SBLK                # 32 blocks per sim row
F_BUFS = 4
P_BUFS = 2
O_BUFS = 2

_cache: dict = {}


def _install_axon_ntff_hook():
    """The agent image's antenv lacks axon_hooks; synthesize it so
    run_bass_kernel_spmd(trace=True) can capture NTFF profiles."""
    try:
        from antenv.axon_hooks import get_axon_ntff_profile_hook  # noqa: F401
        return
    except ImportError:
        pass
    try:
        from trn_agent_boot.trn_boot import _ntff_profile_via_ctypes
        hook = _ntff_profile_via_ctypes("/opt/axon/libaxon_pjrt.so")
    except Exception:
        hook = None
    mod = types.ModuleType("antenv.axon_hooks")
    mod.get_axon_ntff_profile_hook = lambda: hook
    mod.set_axon_ntff_profile_hook = lambda h: None
    sys.modules["antenv.axon_hooks"] = mod


def build():
    nc = bacc.Bacc("TRN2", target_bir_lowering=False, debug=False,
                   num_devices=NCORES)

    emb = nc.dram_tensor("emb", [N, LD], F32, kind="ExternalInput")
    simb = nc.dram_tensor("simb", [RPD * NBLK, SBLK], F32, kind="ExternalInput")
    # per-k sim tables (16 wrapped cols each, both chunks) then per-(c,k)
    # feature tables (8 wrapped cols each)
    gidx = nc.dram_tensor("gidx", [128, K * 16 + NCH * K * 8], I16,
                          kind="ExternalInput")
    rmod = nc.dram_tensor("rmod", [128, NCH * K], F32, kind="ExternalInput")
    iotac = nc.dram_tensor("iotac", [128, SBLK], F32, kind="ExternalInput")
    wout = nc.dram_tensor("wout", [RPD, K], F32, kind="ExternalOutput")
    eout = nc.dram_tensor("eout", [RPD, LD], F32, kind="ExternalOutput")

    with tile.TileContext(nc) as tc:
        with (
            tc.tile_pool(name="const", bufs=1) as cpool,
            tc.tile_pool(name="bblk", bufs=K) as bpool,
            tc.tile_pool(name="hh", bufs=2) as hpool,
            tc.tile_pool(name="sm", bufs=2) as smpool,
            tc.tile_pool(name="stat", bufs=8) as stpool,
            tc.tile_pool(name="feat", bufs=F_BUFS) as fpool,
            tc.tile_pool(name="prod", bufs=P_BUFS) as ppool,
            tc.tile_pool(name="acc", bufs=O_BUFS) as apool,
        ):
            gidx_t = cpool.tile([128, K * 16 + NCH * K * 8], I16)
            nc.sync.dma_start(gidx_t[:], gidx[:])
            rmod_t = cpool.tile([128, NCH * K], F32)
            nc.sync.dma_start(rmod_t[:], rmod[:])
            iota_t = cpool.tile([128, SBLK], F32)
            nc.sync.dma_start(iota_t[:], iotac[:])

            nc.gpsimd.load_library(library_config.mlp)

            # sim-block gathers: one per k covering both chunks (256 idxs)
            bts = []
            for k in range(K):
                bt = bpool.tile([128, NCH, SBLK], F32)
                nc.gpsimd.dma_gather(
                    bt[:], simb[:], gidx_t[:, k * 16:(k + 1) * 16],
                    NCH * 128, NCH * 128, SBLK)
                bts.append(bt)

            fts = {}
            for c in range(NCH):
                for k in range(K):
                    ci = K * 16 + (c * K + k) * 8
                    ft = fpool.tile([128, 1, LD], F32)
                    nc.gpsimd.dma_gather(
                        ft[:], emb[:], gidx_t[:, ci:ci + 8], 128, 128, LD)
                    fts[(c, k)] = ft

            for c in range(NCH):
                # --- sub-similarity extraction ---
                sub = smpool.tile([128, K], F32)
                for k in range(K):
                    j = c * K + k
                    h = hpool.tile([128, SBLK], F32, tag="h")
                    # h = (iota == rmod) * 2   (folds the 1/T scale)
                    nc.vector.tensor_scalar(
                        out=h[:], in0=iota_t[:], scalar1=rmod_t[:, j:j + 1],
                        scalar2=2.0, op0=mybir.AluOpType.is_equal,
                        op1=mybir.AluOpType.mult)
                    p = hpool.tile([128, SBLK], F32, tag="p")
                    nc.vector.tensor_tensor(
                        out=p[:], in0=h[:], in1=bts[k][:, c, :],
                        op=mybir.AluOpType.mult)
                    nc.vector.reduce_sum(
                        out=sub[:, k:k + 1], in_=p[:],
                        axis=mybir.AxisListType.X)

                # --- softmax over K ---
                negmx = stpool.tile([128, 1], F32, tag="negmx")
                nc.vector.reduce_max(out=negmx[:], in_=sub[:],
                                     axis=mybir.AxisListType.X, negate=True)
                e = smpool.tile([128, K], F32, tag="e")
                ssum = stpool.tile([128, 1], F32, tag="ssum")
                nc.scalar.activation(
                    out=e[:], in_=sub[:],
                    func=mybir.ActivationFunctionType.Exp,
                    bias=negmx[:], scale=1.0, accum_out=ssum[:])
                rcp = stpool.tile([128, 1], F32, tag="rcp")
                nc.vector.reciprocal(rcp[:], ssum[:])
                w = smpool.tile([128, K], F32, tag="w")
                nc.vector.tensor_scalar(
                    out=w[:], in0=e[:], scalar1=rcp[:, 0:1], scalar2=None,
                    op0=mybir.AluOpType.mult)
                nc.sync.dma_start(wout[c * 128:(c + 1) * 128, :], w[:])

                # --- weighted sum of neighbor features (ACT mult, DVE add) ---
                acc = apool.tile([128, LD], F32)
                nc.scalar.activation(
                    out=acc[:], in_=fts[(c, 0)][:, 0, :],
                    func=mybir.ActivationFunctionType.Copy,
                    scale=w[:, 0:1])
                for k in range(1, K):
                    pk = ppool.tile([128, LD], F32)
                    nc.scalar.activation(
                        out=pk[:], in_=fts[(c, k)][:, 0, :],
                        func=mybir.ActivationFunctionType.Copy,
                        scale=w[:, k:k + 1])
                    nc.vector.tensor_tensor(
                        out=acc[:], in0=acc[:], in1=pk[:],
                        op=mybir.AluOpType.add)

                nc.sync.dma_start(eout[c * 128:(c + 1) * 128, :], acc[:])

    nc.compile()
    return nc


def prep_inputs(similarity_matrix, batch_emb_om, index):
    """Shard + marshal full inputs into per-core input maps."""
    sim = np.ascontiguousarray(similarity_matrix, dtype=np.float32)
    emb = np.ascontiguousarray(
        np.asarray(batch_emb_om, dtype=np.float32).reshape(N, LD))
    idx = np.asarray(index, dtype=np.int64).astype(np.int32)

    iotac = np.tile(np.arange(SBLK, dtype=np.float32), (128, 1))

    def wrap(lst):
        # dma_gather order: item i read from idxs[i%16, i//16]; replicate to
        # all 8 GPSIMD core groups (16 partitions each).
        return np.tile(lst.astype(np.int16).reshape(-1, 16).T, (8, 1))

    in_maps = []
    for d in range(NCORES):
        idx_d = idx[d * RPD:(d + 1) * RPD]            # [256, K]
        gidx_d = np.zeros((128, K * 16 + NCH * K * 8), dtype=np.int16)
        rmod_d = np.zeros((128, NCH * K), dtype=np.float32)
        rows_all = np.arange(RPD)
        for k in range(K):
            cols = idx_d[rows_all, k]
            sim_lst = (rows_all * NBLK + (cols >> 6))  # [256]
            gidx_d[:, k * 16:(k + 1) * 16] = wrap(sim_lst)
        for c in range(NCH):
            rows = np.arange(c * 128, (c + 1) * 128)
            for k in range(K):
                cols = idx_d[rows, k]
                ci = K * 16 + (c * K + k) * 8
                gidx_d[:, ci:ci + 8] = wrap(cols)
                rmod_d[:, c * K + k] = (cols & 63).astype(np.float32)
        in_maps.append({
            "emb": emb,
            "simb": sim[d * RPD:(d + 1) * RPD].reshape(RPD * NBLK, SBLK),
            "gidx": gidx_d,
            "rmod": rmod_d,
            "iotac": iotac,
        })
    return in_maps


def assemble_outputs(results):
    wfull = np.concatenate([r["wout"] for r in results], axis=0)
    efull = np.concatenate([r["eout"] for r in results], axis=0)
    return wfull.reshape(N, K), efull.reshape(N, L, D)


def kernel(similarity_matrix, batch_emb_om, index, trace=False):
    if trace:
        _install_axon_ntff_hook()
    if "nc" not in _cache:
        _cache["nc"] = build()
    nc = _cache["nc"]
    in_maps = prep_inputs(similarity_matrix, batch_emb_om, index)
    res = run_bass_kernel_spmd(nc, in_maps, core_ids=list(range(NCORES)),
                               trace=trace)
    out = assemble_outputs(res.results)
    if trace:
        _cache["last_result"] = res
    return out
